# revision 33
# baseline (speedup 1.0000x reference)
"""Trainium2 Bass kernel: 5x5 window median+variance denoise filter.

y = relu(x - noise_var/(var5x5(x)+1e-10) * (x - median5x5(x) + noise_bias))
with zero-padded 5x5 windows, unbiased variance (ddof=1).

Sharding: pure data parallel, B=16 images split 2-per-core across 8 cores.

v3 design (fp16 median + multi-engine variance):
- Host casts x to fp16 (padded). The median of fp16-rounded values is the
  fp16 rounding of the true median (rounding is monotone), so the whole
  comparator network runs in fp16 at the DVE 2x tensor_tensor rate.
  (The Pool/gpsimd engine cannot legally execute TensorTensor on TRN2 --
  the ISA check rejects it -- so the network runs entirely on DVE.)
- Variance: vertical 5-row sums of x and gamma*x^2 via banded matmuls on
  the otherwise-idle PE engine (fp16 in, exact fp32 PSUM accumulation);
  horizontal 5-col sums via fp32 prefix scans + prefix differences (the
  shared prefix rounding error cancels in the difference; per-sub-row 2D
  slice APs -- flattened/rearranged APs break tile dependency tracking).
  gamma = 1/(24*noise_var) is folded host-side so the reciprocal directly
  yields nv/(var+1e-10): rcp = 1/(gamma*q25 + gamma*24e-10 - gamma*s25^2/25).
- Activation engine: x^2 (scaled by sqrt(gamma)), PSUM evictions, final
  relu (with scale=-1 folding the sign of the pre-relu value), and half
  the input DMA queue. The variance chain is emitted mid-network (after
  sort5) so the DVE queue head never waits on the ACT/PE producers.
- sv/qv and the PSUM tensors are sequenced per image: emission order IS
  dependency order for shared buffers.
HW-validated vs the jax reference: rel err 1.6e-3 (harness gate 2e-2).
HW exec ~380 us/core/pass (vs 556 us simulated for the fp32 baseline).

Median comparator network (pruned, shared column sorts):
  sort5 over the 5 dy-shifted planes (9 CE, shared by 5 horizontal windows)
  T = odd-even merge of adjacent sorted columns (13 CE, shared by 2 windows)
  final rank-12 selection from T(x-2), T(x), S(x+2) (35 CE, single-sided
  min/max pruned) -- verified offline by exhaustive 0-1 principle.
"""
import numpy as np

import concourse.bass as bass  # noqa: F401
import concourse.mybir as mybir
from concourse import bacc, tile
from concourse.bass_utils import run_bass_kernel_spmd

F32 = mybir.dt.float32
F16 = mybir.dt.float16
ALU = mybir.AluOpType
ACTF = mybir.ActivationFunctionType

# (i, j, need_min, need_max) per structure; designed + 0/1-verified offline.
SORT5 = [(0, 1, 1, 1), (3, 4, 1, 1), (2, 4, 1, 1), (2, 3, 1, 1), (0, 3, 1, 1),
         (0, 2, 1, 1), (1, 4, 1, 1), (1, 3, 1, 1), (1, 2, 1, 1)]
T_CES = [(0, 5, 1, 1), (4, 9, 1, 1), (4, 5, 1, 1), (2, 7, 1, 1), (2, 4, 1, 1),
         (7, 5, 1, 1), (1, 6, 1, 1), (3, 8, 1, 1), (3, 6, 1, 1), (1, 2, 1, 1),
         (3, 4, 1, 1), (6, 7, 1, 1), (8, 5, 1, 1)]
F_CES = [(0, 10, 0, 1), (5, 15, 1, 0), (5, 10, 1, 1), (4, 14, 1, 1),
         (4, 5, 0, 1), (14, 10, 1, 0), (2, 12, 0, 1), (7, 17, 1, 0),
         (7, 12, 1, 1), (7, 5, 0, 1), (12, 14, 1, 1), (1, 11, 0, 1),
         (9, 19, 1, 0), (9, 11, 1, 1), (6, 16, 1, 1), (6, 9, 0, 1),
         (16, 11, 1, 0), (3, 13, 0, 1), (8, 18, 1, 0), (8, 13, 1, 1),
         (8, 9, 1, 1), (13, 16, 1, 0), (8, 5, 1, 1), (9, 12, 1, 1),
         (13, 14, 1, 1), (8, 20, 0, 1), (13, 24, 1, 0), (13, 20, 0, 1),
         (9, 22, 0, 1), (22, 20, 1, 0), (5, 21, 0, 1), (14, 21, 1, 0),
         (12, 23, 1, 0), (12, 14, 0, 1), (14, 22, 1, 0)]
F_OUT = 14

H = 512
W = 512
IMGS_PER_CORE = 2
N_CORES = 8
WIDE = W + 4          # 2-col halo each side
ROWS = 4              # sub-rows per partition: one chunk = one full image
SW = WIDE + 1         # scan row width (leading zero column)
CSPLIT = 352          # DVE strip [0, CSPLIT), Pool strip [CSPLIT, W)
NBUF16D = 26          # fp16 pool cap, DVE strip
NBUF16P = 26          # fp16 pool cap, Pool strip
NBUF32 = 4            # fp32 pool cap


class BufPool:
    """Free-list over preallocated fixed SBUF tensors. Tile's dependency
    tracker makes reuse safe (WAR/RAW serialization on the same tensor).
    Lent buffers (from lend()) may be wider than `shape`; ops only address
    the leading columns they need."""

    def __init__(self, nc, tag, dt, shape, cap):
        self.nc = nc
        self.tag = tag
        self.dt = dt
        self.shape = shape
        self.cap = cap
        self.bufs = []
        self.free = []
        self.peak = 0

    def alloc(self):
        if self.free:
            return self.free.pop()
        idx = len(self.bufs)
        assert idx < self.cap, f"SBUF pool {self.tag} exhausted"
        t = self.nc.alloc_sbuf_tensor(f"{self.tag}{idx}", self.shape,
                                      self.dt).ap()
        self.bufs.append(t)
        self.peak = max(self.peak, len(self.bufs))
        return t

    def release(self, t):
        self.free.append(t)

    def lend(self, t):
        """Add a foreign (same-or-wider) buffer to the free list. Inserted
        at the bottom so it is only used under peak pressure, by which time
        the lender's readers have long drained (cheap WAR)."""
        self.free.insert(0, t)

    def reclaim(self, aps):
        ids = set(id(t) for t in aps)
        self.free = [b for b in self.free if id(b) not in ids]


class Wire:
    """SSA value living at column offset `off` of `buf`."""

    def __init__(self, buf, off, owned, pool, on_die=None):
        self.buf = buf
        self.off = off
        self.owned = owned
        self.pool = pool
        self.reads_left = 0
        self.on_die = on_die

    def ap(self, width):
        return self.buf[:, :, self.off:self.off + width]

    def read_done(self):
        self.reads_left -= 1
        if self.reads_left == 0:
            self._die()

    def read_done_zero(self):
        if self.reads_left == 0:
            self._die()

    def _die(self):
        if self.owned:
            self.pool.release(self.buf)
        if self.on_die is not None:
            self.on_die()

    def detach_views(self, n_views):
        buf, owned, pool = self.buf, self.owned, self.pool
        self.owned = False
        state = {"n": n_views}

        def on_die():
            state["n"] -= 1
            if state["n"] == 0 and owned:
                pool.release(buf)
        return on_die


def run_stage(nc, eng, pool, wires, ces, width, terminal_reads):
    """Emit one comparator-network stage on a single engine."""
    n = len(wires)
    segs = [[] for _ in range(n)]
    cur = [0] * n
    for (a, b, nmin, nmax) in ces:
        cur[a] += 1
        cur[b] += 1
        if nmin:
            segs[a].append(cur[a])
            cur[a] = 0
        if nmax:
            segs[b].append(cur[b])
            cur[b] = 0
    for i in range(n):
        segs[i].append(cur[i] + terminal_reads.get(i, 0))

    seg_idx = [0] * n
    for i in range(n):
        wires[i].reads_left += segs[i][0]
        if segs[i][0] == 0:
            wires[i].read_done_zero()

    for (i, j, nmin, nmax) in ces:
        wi, wj = wires[i], wires[j]
        a = wi.ap(width)
        b = wj.ap(width)
        if nmin:
            lo = pool.alloc()
            eng.tensor_tensor(lo[:, :, 0:width], a, b, ALU.min)
        if nmax:
            hi = pool.alloc()
            eng.tensor_tensor(hi[:, :, 0:width], a, b, ALU.max)
        wi.read_done()
        wj.read_done()
        if nmin:
            seg_idx[i] += 1
            cnt = segs[i][seg_idx[i]]
            assert cnt > 0, "dead write (should be pruned offline)"
            wires[i] = Wire(lo, 0, True, pool)
            wires[i].reads_left = cnt
        if nmax:
            seg_idx[j] += 1
            cnt = segs[j][seg_idx[j]]
            assert cnt > 0, "dead write (should be pruned offline)"
            wires[j] = Wire(hi, 0, True, pool)
            wires[j].reads_left = cnt


def emit_strip(nc, eng, pool, st, img, c0, n, mid_emit=None):
    """Median network + formula for output columns [c0, c0+n) on `eng`.
    `mid_emit` (if given) is called after the T stage to interleave other
    same-engine work into the queue at a point where its cross-engine
    dependencies have long been satisfied."""
    tin = st["tin"][img]
    tv = [tin[k][:, :, c0:c0 + n + 4] for k in range(5)]

    s_wires = [Wire(tv[k], 0, False, pool) for k in range(5)]
    run_stage(nc, eng, pool, s_wires, SORT5, n + 4, {k: 1 for k in range(5)})

    if mid_emit is not None:
        mid_emit()

    t_wires = [None] * 10
    c_views = [None] * 5
    for k in range(5):
        rk = s_wires[k]
        od = rk.detach_views(3)
        t_wires[k] = Wire(rk.buf, rk.off + 0, False, pool, on_die=od)
        t_wires[k + 5] = Wire(rk.buf, rk.off + 1, False, pool, on_die=od)
        c_views[k] = Wire(rk.buf, rk.off + 4, False, pool, on_die=od)
        rk.read_done()

    run_stage(nc, eng, pool, t_wires, T_CES, n + 3, {j: 1 for j in range(10)})

    f_wires = [None] * 25
    for j in range(10):
        tw = t_wires[j]
        od = tw.detach_views(2)
        f_wires[j] = Wire(tw.buf, tw.off + 0, False, pool, on_die=od)
        f_wires[j + 10] = Wire(tw.buf, tw.off + 2, False, pool, on_die=od)
        tw.read_done()
    for k in range(5):
        f_wires[20 + k] = c_views[k]

    run_stage(nc, eng, pool, f_wires, F_CES, n, {F_OUT: 1})
    mid = f_wires[F_OUT]
    if img == 0 and st.get("taps"):
        mid.reads_left += 1
        nc.sync.dma_start(st["taps"]["t_mid"].ap()[:, :, 0:n], mid.ap(n))
        mid.read_done()

    # ---- formula: y = relu((x - nb*rcp) - rcp*(x - mid)) ----
    # all three ops are fp16 tensor_tensor (DVE 2x rate); xr = x - nb*rcp
    # is precomputed once per chunk. The pre-relu result goes to a
    # dedicated buffer: if it lived in pool scratch, the (late-running)
    # ACT relu's read would WAR-block the next image's network ops that
    # reuse the buffer.
    xc = tin[2][:, :, c0 + 2:c0 + 2 + n]
    rv = st["rcp32_live"][:, :, c0:c0 + n]
    nxr = st["xrn"][img][:, :, c0:c0 + n]   # nb*rcp - x
    vres = st["vres"][img][0]
    u = pool.alloc()
    eng.tensor_tensor(u[:, :, 0:n], xc, mid.ap(n), ALU.subtract)
    mid.read_done()
    eng.tensor_tensor(u[:, :, 0:n], rv, u[:, :, 0:n], ALU.mult)
    eng.tensor_tensor(vres[:, :, 0:n], nxr, u[:, :, 0:n], ALU.add)
    pool.release(u)
    nc.scalar.activation(st["out"][img][:, :, c0:c0 + n], vres[:, :, 0:n],
                         ACTF.Relu, scale=-1.0)
    # per-strip store so neither strip's tail blocks the other
    ya = st["ya"]
    dst = ya[img * H: img * H + H, c0:c0 + n].rearrange(
        "(a p) c -> p a c", p=128)
    nc.sync.dma_start(dst, st["out"][img][:, :, c0:c0 + n])


def emit_loads(nc, pools, st, img):
    """DMA the 5 dy-shifted fp16 planes; split per plane across the SP and
    ACT HW-DGE queues so the first sort5 inputs land early."""
    pool16d, _, _ = pools
    tin = st["tin"][img]
    xa = st["xa"]
    pool16d.reclaim(tin + [st["sq"][img]])
    for k in (0, 1, 3, 4, 2):
        s = img * (H + 4) + (k - 2) + 2
        lo = xa[s: s + 256, :].rearrange("(a p) c -> p a c", p=128)
        hi = xa[s + 256: s + 512, :].rearrange("(a p) c -> p a c", p=128)
        nc.sync.dma_start(tin[k][:, 0:2, :], lo)
        nc.scalar.dma_start(tin[k][:, 2:4, :], hi)


def emit_vertical(nc, st, img):
    """gamma-scaled square (ACT) + vertical 5-sums (PE banded matmuls) +
    PSUM evictions (ACT)."""
    tin = st["tin"][img]
    sv, qv = st["sv"][img], st["qv"][img]
    xc_plane = tin[2]                     # rows 0..511, dy=0
    sq = st["sq"][img]
    nc.scalar.activation(sq[:, :, :], xc_plane[:, :, :], ACTF.Square,
                         scale=st["sqrtg_ap"])

    band, fxl, fxh = st["band"], st["fxl"], st["fxh"]
    ps, pq = st["psum_s"], st["psum_q"]
    for src, psum in ((xc_plane, ps), (sq, pq)):
        for b in range(ROWS):
            mm = [(band, src[:, b, 0:512], True)]
            if b > 0:
                mm.append((fxl, src[:, b - 1, 0:512], False))
            if b < ROWS - 1:
                mm.append((fxh, src[:, b + 1, 0:512], False))
            for q, (lhsT, rhs, is_first) in enumerate(mm):
                nc.tensor.matmul(
                    psum[:, b, :], lhsT, rhs,
                    start=is_first, stop=(q == len(mm) - 1))

    # sv/qv layout: [128, ROWS, SW]; col 0 is a (one-time memset) zero,
    # cols 1..SW-1 hold the 516 vertical sums. (img1's evictions WAR-wait
    # on img0's scans -- by then ACT has nothing better to do anyway.)
    nc.scalar.activation(sv[:, :, 1:513], ps[:, :, 0:512], ACTF.Copy)
    nc.scalar.activation(qv[:, :, 1:513], pq[:, :, 0:512], ACTF.Copy)


def emit_variance_dve(nc, pools, st, img):
    """The DVE/fp32 part of the variance chain: right-halo vertical sums,
    prefix scans, prefix differences, reciprocal, and the nb*rcp - x
    plane for the formula. Emitted mid-strip via mid_emit so the DVE
    queue-head never waits on the ACT/PE producers."""
    _, _, pool32 = pools
    tin = st["tin"][img]
    sv, qv = st["sv"][img], st["qv"][img]

    # right-halo cols 512..515: tiny vertical sums
    e = slice(512, 516)
    nc.vector.tensor_tensor(sv[:, :, 513:517], tin[0][:, :, e],
                            tin[1][:, :, e], ALU.add)
    for k in (2, 3, 4):
        nc.vector.tensor_tensor(sv[:, :, 513:517], sv[:, :, 513:517],
                                tin[k][:, :, e], ALU.add)
    sqe = pool32.alloc()
    nc.scalar.activation(qv[:, :, 513:517], tin[0][:, :, e], ACTF.Square,
                         scale=st["sqrtg_ap"])
    for k in (1, 2, 3, 4):
        nc.scalar.activation(sqe[:, :, 0:4], tin[k][:, :, e], ACTF.Square,
                             scale=st["sqrtg_ap"])
        nc.vector.tensor_tensor(qv[:, :, 513:517], qv[:, :, 513:517],
                                sqe[:, :, 0:4], ALU.add)
    pool32.release(sqe)

    if img == 0 and st.get("taps"):
        # same-engine snapshot of sv/qv exactly as the scans will read them
        nc.vector.tensor_tensor(st["tapbuf_s"][:, :, :], sv[:, :, :],
                                sv[:, :, :], ALU.min)
        nc.vector.tensor_tensor(st["tapbuf_q"][:, :, :], qv[:, :, :],
                                qv[:, :, :], ALU.min)
        nc.sync.dma_start(st["taps"]["t_sv"].ap()[:, :, :],
                          st["tapbuf_s"][:, :, :])
        nc.sync.dma_start(st["taps"]["t_qv"].ap()[:, :, :],
                          st["tapbuf_q"][:, :, :])

    # horizontal 5-sums via fp32 prefix scan + difference; one scan per
    # sub-row (2D slice APs -- rearranged/flattened APs break the tile
    # dependency tracker, which raced img1's evictions against these reads)
    zb = st["zb"]
    p_s = pool32.alloc()
    for b in range(ROWS):
        nc.vector.tensor_tensor_scan(
            p_s[:, b, :], sv[:, b, :], zb, 0.0, ALU.add, ALU.add)
    s25 = pool32.alloc()
    nc.vector.tensor_tensor(s25[:, :, 0:W], p_s[:, :, 5:5 + W],
                            p_s[:, :, 0:W], ALU.subtract)
    pool32.release(p_s)
    p_q = pool32.alloc()
    for b in range(ROWS):
        nc.vector.tensor_tensor_scan(
            p_q[:, b, :], qv[:, b, :], zb, 0.0, ALU.add, ALU.add)
    # q25b = (gamma*q25[x+5] + gamma*24e-10) - gamma*q25[x]
    q25 = pool32.alloc()
    nc.vector.scalar_tensor_tensor(q25[:, :, 0:W], p_q[:, :, 5:5 + W],
                                   st["gb_ap"], p_q[:, :, 0:W], ALU.add,
                                   ALU.subtract)
    pool32.release(p_q)
    # rcp_in = q25b - (gamma/25)*s25^2 ; rcp = nv/(var+1e-10)
    # (the square runs on the idle ACT engine; it parks in the DVE wait
    # queue without stalling the F-stage thanks to the lookahead window)
    m = pool32.alloc()
    nc.scalar.activation(m[:, :, 0:W], s25[:, :, 0:W], ACTF.Square)
    pool32.release(s25)
    nc.vector.scalar_tensor_tensor(m[:, :, 0:W], m[:, :, 0:W],
                                   st["gm_ap"], q25[:, :, 0:W], ALU.mult,
                                   ALU.add)
    pool32.release(q25)
    rcp32 = pool32.alloc()
    nc.vector.reciprocal_approx_fast(rcp32[:, :, 0:W], m[:, :, 0:W])
    pool32.release(m)
    st["rcp32_live"] = rcp32     # released after both strips' formulas
    if img == 0 and st.get("taps"):
        tp = st["taps"]
        nc.vector.tensor_tensor(st["tapbuf_s"][:, :, :], rcp32[:, :, :],
                                rcp32[:, :, :], ALU.min)
        nc.sync.dma_start(tp["t_rcp"].ap()[:, :, :],
                          st["tapbuf_s"][:, :, :])
        nc.sync.dma_start(tp["t_xrn"].ap()[:, :, :], st["xrn"][img][:, :, :])
        for k in range(5):
            nc.sync.dma_start(tp["t_tin"].ap()[k, :, :, :], tin[k][:, :, :])
    # xrn = nb*rcp - x (consumed by both strips' formula tails)
    nc.vector.scalar_tensor_tensor(st["xrn"][img][:, :, :],
                                   rcp32[:, :, 0:W], st["nb_ap"],
                                   tin[2][:, :, 2:2 + W], ALU.mult,
                                   ALU.subtract)


def emit_chunk(nc, pools, st, img):
    """One chunk = one full image: [128, ROWS, *] tiles, image row =
    128*b + partition. Loads + ACT/PE vertical work were already emitted
    in an earlier phase for both images."""
    pool16d, pool16p, pool32 = pools
    tin = st["tin"][img]

    emit_strip(nc, nc.vector, pool16d, st, img, 0, W,
               mid_emit=lambda: emit_variance_dve(nc, pools, st, img))
    pool32.release(st.pop("rcp32_live"))

    # tin planes (except center) and sq are dead; lend as DVE scratch
    for k in (0, 1, 3, 4):
        pool16d.lend(tin[k])
    pool16d.lend(st["sq"][img])


def build_module(hw_loop=None, taps=False):
    nc = bacc.Bacc(
        "TRN2",
        target_bir_lowering=False,
        debug=False,
        enable_asserts=False,
        num_devices=N_CORES,
    )
    x = nc.dram_tensor("x", [IMGS_PER_CORE, H + 4, WIDE], F16,
                       kind="ExternalInput")
    nvb16 = nc.dram_tensor("nvb16", [128, 1], F16, kind="ExternalInput")
    nvb32 = nc.dram_tensor("nvb32", [128, 4], F32, kind="ExternalInput")
    band_d = nc.dram_tensor("band", [128, 128], F16, kind="ExternalInput")
    fix_d = nc.dram_tensor("fix", [128, 256], F16, kind="ExternalInput")
    y = nc.dram_tensor("y", [IMGS_PER_CORE, H, W], F32, kind="ExternalOutput")
    tap_t = {}
    if taps:
        for name, shape, dt in (
                ("t_tin", [5, 128, ROWS, WIDE], F16),
                ("t_sv", [128, ROWS, SW], F32),
                ("t_qv", [128, ROWS, SW], F32),
                ("t_rcp", [128, ROWS, SW], F32),
                ("t_mid", [128, ROWS, W], F16),
                ("t_xrn", [128, ROWS, W], F16)):
            tap_t[name] = nc.dram_tensor(name, shape, dt,
                                         kind="ExternalOutput")

    xa = x.ap().flatten_outer_dims()    # [2*516, 516] fp16
    ya = y.ap().flatten_outer_dims()

    with tile.TileContext(nc) as tc:
        pool16d = BufPool(nc, "wd", F16, [128, ROWS, WIDE], NBUF16D)
        pool16p = BufPool(nc, "wp", F16, [128, ROWS, 8], NBUF16P)
        pool32 = BufPool(nc, "vb", F32, [128, ROWS, SW], NBUF32)

        nvb16_t = nc.alloc_sbuf_tensor("nvb16_t", [128, 1], F16).ap()
        nc.sync.dma_start(nvb16_t[:, :], nvb16.ap()[:, :])
        nvb32_t = nc.alloc_sbuf_tensor("nvb32_t", [128, 4], F32).ap()
        nc.sync.dma_start(nvb32_t[:, :], nvb32.ap()[:, :])
        band_t = nc.alloc_sbuf_tensor("band_t", [128, 128], F16).ap()
        nc.sync.dma_start(band_t[:, :], band_d.ap()[:, :])
        fix_t = nc.alloc_sbuf_tensor("fix_t", [128, 256], F16).ap()
        nc.sync.dma_start(fix_t[:, :], fix_d.ap()[:, :])
        zcol = nc.alloc_sbuf_tensor("zcol", [128, 1], F32).ap()
        nc.gpsimd.memset(zcol[:, :], 0.0)

        st = {
            "xa": xa, "ya": ya,
            "nb_ap": nvb16_t[:, 0:1],
            "sqrtg_ap": nvb32_t[:, 0:1], "gm_ap": nvb32_t[:, 1:2],
            "gb_ap": nvb32_t[:, 2:3], "g_ap": nvb32_t[:, 3:4],
            "band": band_t, "fxl": fix_t[:, 0:128], "fxh": fix_t[:, 128:256],
            "zb": zcol.broadcast_to([128, SW]),
            "tin": [[nc.alloc_sbuf_tensor(f"tin{i}_{k}", [128, ROWS, WIDE],
                                          F16).ap() for k in range(5)]
                    for i in range(IMGS_PER_CORE)],
            "sq": [nc.alloc_sbuf_tensor(f"sq{i}", [128, ROWS, WIDE], F16).ap()
                   for i in range(IMGS_PER_CORE)],
            "sv": [nc.alloc_sbuf_tensor(f"sv{i}", [128, ROWS, SW], F32).ap()
                   for i in range(IMGS_PER_CORE)],
            "qv": [nc.alloc_sbuf_tensor(f"qv{i}", [128, ROWS, SW], F32).ap()
                   for i in range(IMGS_PER_CORE)],
            "vres": [[nc.alloc_sbuf_tensor(f"vres{i}_0", [128, ROWS, W],
                                           F16).ap()]
                     for i in range(IMGS_PER_CORE)],
            "xrn": [nc.alloc_sbuf_tensor(f"xrn{i}", [128, ROWS, W],
                                         F16).ap()
                    for i in range(IMGS_PER_CORE)],
            "out": [nc.alloc_sbuf_tensor(f"out{i}", [128, ROWS, W], F32).ap()
                    for i in range(IMGS_PER_CORE)],
            "psum_s": nc.alloc_psum_tensor("psum_s", [128, ROWS, 512],
                                           F32).ap(),
            "psum_q": nc.alloc_psum_tensor("psum_q", [128, ROWS, 512],
                                           F32).ap(),
        }

        # one-time zeroing of the leading zero column of sv/qv
        for i in range(IMGS_PER_CORE):
            nc.gpsimd.memset(st["sv"][i][:, :, 0:1], 0.0)
            nc.gpsimd.memset(st["qv"][i][:, :, 0:1], 0.0)

        pools = (pool16d, pool16p, pool32)
        st["taps"] = tap_t
        if tap_t:
            st["tapbuf_s"] = nc.alloc_sbuf_tensor(
                "tapbuf_s", [128, ROWS, SW], F32).ap()
            st["tapbuf_q"] = nc.alloc_sbuf_tensor(
                "tapbuf_q", [128, ROWS, SW], F32).ap()

        def body():
            for img in range(IMGS_PER_CORE):
                emit_loads(nc, pools, st, img)
            for img in range(IMGS_PER_CORE):
                emit_vertical(nc, st, img)
            for img in range(IMGS_PER_CORE):
                emit_chunk(nc, pools, st, img)

        if hw_loop is None:
            body()
        else:
            with tc.For_i(0, hw_loop, 1):
                body()

    nc.compile()
    return nc


_MODULE = None


def _get_module():
    global _MODULE
    if _MODULE is None:
        _MODULE = build_module()
    return _MODULE


def _host_inputs(x, noise_var, noise_bias):
    nv = float(np.asarray(noise_var).reshape(-1)[0])
    nb = float(np.asarray(noise_bias).reshape(-1)[0])
    B = x.shape[0]
    assert x.shape == (B, 1, H, W) and B == N_CORES * IMGS_PER_CORE

    gamma = 1.0 / (24.0 * nv)
    nvb16 = np.full((128, 1), np.float16(nb), np.float16)
    nvb32 = np.empty((128, 4), np.float32)
    nvb32[:, 0] = np.sqrt(gamma)         # ACT square scale
    nvb32[:, 1] = -gamma / 25.0          # s25^2 coefficient
    nvb32[:, 2] = gamma * 24e-10         # epsilon term
    nvb32[:, 3] = gamma                  # right-halo q scale

    band = np.zeros((128, 128), np.float16)
    for k in range(128):
        band[k, max(0, k - 2):k + 3] = 1.0
    fix = np.zeros((128, 256), np.float16)
    # fxl = fix[:, 0:128]: prev block rows 126/127 feed out rows 0/1
    fix[126, 0] = 1.0                    # row -2 -> out row 0
    fix[127, 0] = 1.0                    # row -1 -> out row 0
    fix[127, 1] = 1.0                    # row -1 -> out row 1
    # fxh = fix[:, 128:256]: next block rows 0/1 feed out rows 126/127
    fix[0, 128 + 126] = 1.0              # row +128 -> out row 126
    fix[0, 128 + 127] = 1.0              # row +128 -> out row 127
    fix[1, 128 + 127] = 1.0              # row +129 -> out row 127

    xpad = np.zeros((B, H + 4, WIDE), np.float16)
    xpad[:, 2:2 + H, 2:2 + W] = x[:, 0].astype(np.float16)
    in_maps = []
    for c in range(N_CORES):
        shard = np.ascontiguousarray(
            xpad[c * IMGS_PER_CORE:(c + 1) * IMGS_PER_CORE])
        in_maps.append({"x": shard, "nvb16": nvb16, "nvb32": nvb32,
                        "band": band, "fix": fix})
    return in_maps


def kernel(x, noise_var, noise_bias):
    x = np.ascontiguousarray(np.asarray(x, dtype=np.float32))
    B = x.shape[0]
    in_maps = _host_inputs(x, noise_var, noise_bias)
    nc = _get_module()
    res = run_bass_kernel_spmd(nc, in_maps, core_ids=list(range(N_CORES)))
    y = np.empty((B, 1, H, W), np.float32)
    for c in range(N_CORES):
        y[c * IMGS_PER_CORE:(c + 1) * IMGS_PER_CORE, 0] = res.results[c]["y"]
    return y
